# revision 11
# baseline (speedup 1.0000x reference)
"""Trainium2 Bass kernel for CausalAnalysisPredictor (gnn_message_passing).

kernel(**inputs) takes the FULL unsharded inputs and returns the FULL
[16384, 51] float32 output. Internally it shards the relation axis across
8 NeuronCores (data-parallel; small weights replicated; per-relation rows
of edge_ctx are gathered on-device via indirect DMA from the replicated
node table).
"""

import os
import sys
import types

import numpy as np

try:
    import concourse  # noqa: F401
except ImportError:  # pragma: no cover
    sys.path.insert(0, "/opt/trn_rl_repo")

import ml_dtypes

import concourse.bass as bass
import concourse.mybir as mybir
import concourse.tile as tile
from concourse import bacc
from concourse.bass import IndirectOffsetOnAxis
from concourse.bass_utils import run_bass_kernel_spmd
from concourse.masks import make_identity

BF16 = mybir.dt.bfloat16
F32 = mybir.dt.float32
I32 = mybir.dt.int32
NPBF16 = ml_dtypes.bfloat16

N_OBJ, N_REL = 4096, 16384
H, P = 512, 4096
NOC, NRC = 151, 51
NCORES = 8
NRELC = N_REL // NCORES  # 2048 relations per core
KC = H // 128            # 4 feat chunks of edge_ctx
KCAT = (2 * H) // 128    # 8 feat chunks of ctx_rep
MO = P // 128            # 32 output-feature chunks
NCH = NRELC // 512       # 4 relation chunks of 512
NG = NRELC // 128        # 16 gather calls per index list
GOFF = 64                # partition offset of the gate/vis/freq lane block

AF = mybir.ActivationFunctionType
ALU = mybir.AluOpType

last_exec_time_ns = None  # set when BASS_KERNEL_TRACE=1


def _register_ntff_hook():
    if "antenv.axon_hooks" in sys.modules:
        return
    hook = None
    try:
        from trn_agent_boot.trn_boot import _ntff_profile_via_ctypes

        hook = _ntff_profile_via_ctypes("/opt/axon/libaxon_pjrt.so")
    except Exception:
        hook = None
    mod = types.ModuleType("antenv.axon_hooks")
    mod.get_axon_ntff_profile_hook = lambda: hook
    mod.set_axon_ntff_profile_hook = lambda h: None
    sys.modules["antenv.axon_hooks"] = mod


_nc_cache = None


def _build():
    global _nc_cache
    if _nc_cache is not None:
        return _nc_cache

    nc = bacc.Bacc("TRN2", target_bir_lowering=False, debug=False, num_devices=NCORES)

    # ---- DRAM parameters (per-core shards / replicated tables) ----
    ectx = nc.declare_dram_parameter("ectx", [N_OBJ, H], BF16, isOutput=False)
    freqp = nc.declare_dram_parameter("freqp", [NOC * NOC, 128], BF16, isOutput=False)
    hidx = nc.declare_dram_parameter("hidx", [128, NG], I32, isOutput=False)
    tidx = nc.declare_dram_parameter("tidx", [128, NG], I32, isOutput=False)
    p0w = nc.declare_dram_parameter("p0w", [128, NG], I32, isOutput=False)
    p1w = nc.declare_dram_parameter("p1w", [128, NG], I32, isOutput=False)
    bboxT = nc.declare_dram_parameter("bboxT", [32, NRELC], BF16, isOutput=False)
    uT = nc.declare_dram_parameter("uT", [P, NRELC], BF16, isOutput=False)
    wcat = nc.declare_dram_parameter("wcat", [128, KCAT * MO * 128], BF16, isOutput=False)
    wspt1 = nc.declare_dram_parameter("wspt1", [32, H], BF16, isOutput=False)
    wspt2 = nc.declare_dram_parameter("wspt2", [MO, 128, KC * 128], BF16, isOutput=False)
    wcg = nc.declare_dram_parameter("wcg", [128, MO * 128], BF16, isOutput=False)
    wvisp = nc.declare_dram_parameter("wvisp", [128, MO * 128], BF16, isOutput=False)
    bcat = nc.declare_dram_parameter("bcat", [128, MO], F32, isOutput=False)
    bs1 = nc.declare_dram_parameter("bs1", [128, KC], F32, isOutput=False)
    bs2 = nc.declare_dram_parameter("bs2", [128, MO], F32, isOutput=False)
    bctx = nc.declare_dram_parameter("bctx", [128, 1], F32, isOutput=False)
    bvg = nc.declare_dram_parameter("bvg", [128, 1], F32, isOutput=False)
    out_t = nc.declare_dram_parameter("out_t", [NRC, NRELC], F32, isOutput=True)

    with tile.TileContext(nc) as tc:
        with (
            tc.tile_pool(name="sbuf", bufs=1) as pool,
            tc.tile_pool(name="psum", bufs=1, space="PSUM") as pp,
        ):
            # ---- resident small tensors ----
            ident = pool.tile([128, 128], BF16)
            make_identity(nc, ident[:])
            hidx_t = pool.tile([128, NG], I32)
            nc.sync.dma_start(hidx_t[:], hidx[:])
            tidx_t = pool.tile([128, NG], I32)
            nc.sync.dma_start(tidx_t[:], tidx[:])
            p0_t = pool.tile([128, NG], I32)
            nc.sync.dma_start(p0_t[:], p0w[:])
            p1_t = pool.tile([128, NG], I32)
            nc.sync.dma_start(p1_t[:], p1w[:])
            bboxT_t = pool.tile([32, NRELC], BF16)
            nc.sync.dma_start(bboxT_t[:], bboxT[:])
            wspt1_t = pool.tile([32, H], BF16)
            nc.sync.dma_start(wspt1_t[:], wspt1[:])
            wcat_sb = pool.tile([128, KCAT, MO, 128], BF16)
            nc.scalar.dma_start(
                wcat_sb[:], wcat[:].rearrange("p (k m c) -> p k m c", k=KCAT, m=MO)
            )
            wcg_t = pool.tile([128, MO, 128], BF16)
            nc.scalar.dma_start(wcg_t[:], wcg[:].rearrange("p (m c) -> p m c", m=MO))
            wvis_t = pool.tile([128, MO, 128], BF16)
            nc.scalar.dma_start(wvis_t[:], wvisp[:].rearrange("p (m c) -> p m c", m=MO))
            bcat_t = pool.tile([128, MO], F32)
            nc.sync.dma_start(bcat_t[:], bcat[:])
            bs1_t = pool.tile([128, KC], F32)
            nc.sync.dma_start(bs1_t[:], bs1[:])
            bs2_t = pool.tile([128, MO], F32)
            nc.sync.dma_start(bs2_t[:], bs2[:])
            bctx_t = pool.tile([128, 1], F32)
            nc.sync.dma_start(bctx_t[:], bctx[:])
            bvg_t = pool.tile([128, 1], F32)
            nc.sync.dma_start(bvg_t[:], bvg[:])

            # fidx = p0*151 + p1 (int32 on DVE)
            fidx_t = pool.tile([128, NG], I32)
            nc.vector.tensor_scalar(
                out=fidx_t[:], in0=p0_t[:], scalar1=NOC, scalar2=None, op0=ALU.mult
            )
            nc.vector.tensor_add(out=fidx_t[:], in0=fidx_t[:], in1=p1_t[:])

            # ---- gathered + transposed activations ----
            # eT[j]: feature-major gathered edge_ctx; j<KC head chunks, j>=KC tail
            eT = [pool.tile([128, NRELC], BF16, tag=f"eT{j}", name=f"eT{j}") for j in range(2 * KC)]
            gfT = pool.tile([128, NRELC], F32)

            # ---- spt1 (bbox only; PE warm-up during gather prologue) ----
            s1T = [pool.tile([128, NRELC], BF16, tag=f"s1T{k}", name=f"s1T{k}") for k in range(KC)]
            for k in range(KC):
                for n in range(NCH):
                    ps = pp.tile([128, 512], F32, tag="work", bufs=2)
                    nc.tensor.matmul(
                        ps[:],
                        wspt1_t[:, k * 128 : (k + 1) * 128],
                        bboxT_t[:, n * 512 : (n + 1) * 512],
                        start=True,
                        stop=True,
                    )
                    nc.scalar.activation(
                        s1T[k][:, n * 512 : (n + 1) * 512],
                        ps[:],
                        AF.Relu,
                        bias=bs1_t[:, k : k + 1],
                    )

            outT = pool.tile([128, NRELC], F32)
            gs = pool.tile([128, NRELC], BF16)

            def gather_block(idx_tile, src_dram, i, base_j, copy_eng):
                """Gather 128 rows (call i) and PE-transpose into eT[base_j+k] cols."""
                g = pool.tile([128, H], BF16, tag="g", bufs=6)
                nc.gpsimd.indirect_dma_start(
                    out=g[:],
                    out_offset=None,
                    in_=src_dram[:],
                    in_offset=IndirectOffsetOnAxis(ap=idx_tile[:, i : i + 1], axis=0),
                )
                for k in range(KC):
                    pt = pp.tile([128, 128], BF16, tag="work", bufs=2)
                    nc.tensor.transpose(pt[:], g[:, k * 128 : (k + 1) * 128], ident[:])
                    if copy_eng == "act":
                        nc.scalar.activation(
                            eT[base_j + k][:, i * 128 : (i + 1) * 128], pt[:], AF.Copy
                        )
                    else:
                        nc.vector.tensor_copy(
                            out=eT[base_j + k][:, i * 128 : (i + 1) * 128], in_=pt[:]
                        )

            for n in range(NCH):
                nsl = slice(n * 512, (n + 1) * 512)
                # -- gathers for this relation chunk (SWDGE queue runs ahead) --
                for i in range(4 * n, 4 * n + 4):
                    gather_block(hidx_t, ectx, i, 0, "act")
                    gather_block(tidx_t, ectx, i, KC, "dve")
                for i in range(4 * n, 4 * n + 4):
                    gf = pool.tile([128, 128], BF16, tag="gf", bufs=4)
                    nc.gpsimd.indirect_dma_start(
                        out=gf[:],
                        out_offset=None,
                        in_=freqp[:],
                        in_offset=IndirectOffsetOnAxis(ap=fidx_t[:, i : i + 1], axis=0),
                    )
                    ptf = pp.tile([128, 128], BF16, tag="work", bufs=2)
                    nc.tensor.transpose(ptf[:], gf[:], ident[:])
                    nc.scalar.activation(gfT[:, i * 128 : (i + 1) * 128], ptf[:], AF.Copy)

                # -- main: post_cat x spt gate -> ctx/gate/vis heads --
                psum_cg = pp.tile([128, 512], F32, tag="cg", name=f"cg{n}", bufs=2)
                lag = []  # (pc, u_b, m) awaiting their cg/vis matmuls
                for m in range(MO + 1):
                    if m < MO:
                        wspt2_b = pool.tile([128, KC * 128], BF16, tag="wspt2_b", bufs=4)
                        nc.scalar.dma_start(wspt2_b[:], wspt2[m])
                        u_b = pool.tile([128, 512], BF16, tag="u_b", bufs=4)
                        nc.sync.dma_start(u_b[:], uT[m * 128 : (m + 1) * 128, nsl])
                        ps_cat = pp.tile([128, 512], F32, tag="cat", bufs=2)
                        for k in range(KCAT):
                            nc.tensor.matmul(
                                ps_cat[:],
                                wcat_sb[:, k, m, :],
                                eT[k][:, nsl],
                                start=(k == 0),
                                stop=(k == KCAT - 1),
                            )
                        ps_spt = pp.tile([128, 512], F32, tag="spt", bufs=2)
                        for k in range(KC):
                            nc.tensor.matmul(
                                ps_spt[:],
                                wspt2_b[:, k * 128 : (k + 1) * 128],
                                s1T[k][:, nsl],
                                start=(k == 0),
                                stop=(k == KC - 1),
                            )
                        r1 = pool.tile([128, 512], BF16, tag="r1", bufs=3)
                        nc.scalar.activation(
                            r1[:], ps_cat[:], AF.Relu, bias=bcat_t[:, m : m + 1]
                        )
                        r2 = pool.tile([128, 512], BF16, tag="r2", bufs=3)
                        nc.vector.tensor_scalar(
                            out=r2[:],
                            in0=ps_spt[:],
                            scalar1=bs2_t[:, m : m + 1],
                            scalar2=0.0,
                            op0=ALU.add,
                            op1=ALU.max,
                        )
                        pc = pool.tile([128, 512], BF16, tag="pc", bufs=3)
                        nc.vector.tensor_mul(out=pc[:], in0=r1[:], in1=r2[:])
                        lag.append((pc, u_b, m))
                    if lag and (len(lag) > 1 or m == MO):
                        pc_l, u_l, m_l = lag.pop(0)
                        nc.tensor.matmul(
                            psum_cg[:],
                            wcg_t[:, m_l, :],
                            pc_l[:],
                            start=(m_l == 0),
                            stop=False,
                            skip_group_check=True,
                        )
                        nc.tensor.matmul(
                            psum_cg[:],
                            wvis_t[:, m_l, :],
                            u_l[:],
                            start=False,
                            stop=(m_l == MO - 1),
                            skip_group_check=True,
                        )

                # -- epilogue: rel^T = (ctx + b_ctx) * sigmoid(vis+gate+frq+b_vg) --
                sarg = pool.tile([128, 512], F32, tag="sarg", bufs=2)
                nc.vector.tensor_add(
                    out=sarg[GOFF : GOFF + NRC, :],
                    in0=psum_cg[GOFF : GOFF + NRC, :],
                    in1=gfT[GOFF : GOFF + NRC, nsl],
                )
                sg = pool.tile([128, 512], BF16, tag="sg", bufs=2)
                nc.scalar.activation(
                    sg[GOFF : GOFF + NRC, :],
                    sarg[GOFF : GOFF + NRC, :],
                    AF.Sigmoid,
                    bias=bvg_t[GOFF : GOFF + NRC, :],
                )
                # shift sigmoid output from partitions GOFF.. to 0..
                nc.sync.dma_start(gs[0:NRC, nsl], sg[GOFF : GOFF + NRC, :])
                nc.vector.scalar_tensor_tensor(
                    out=outT[0:NRC, nsl],
                    in0=psum_cg[0:NRC, :],
                    scalar=bctx_t[0:NRC, :],
                    in1=gs[0:NRC, nsl],
                    op0=ALU.add,
                    op1=ALU.mult,
                )

            nc.sync.dma_start(out_t[:], outT[0:NRC, :])

    nc.compile()
    _nc_cache = nc
    return _nc_cache


def _wrap_idx(idx):
    """[NRELC] -> [128, NG] int32 with idx[i*128+p] at [p, i]."""
    return np.ascontiguousarray(idx.reshape(NG, 128).T.astype(np.int32))


def _prep_core(inputs, c, common):
    sl = slice(c * NRELC, (c + 1) * NRELC)
    pair_idx = np.asarray(inputs["pair_idx"][sl]).astype(np.int64)
    pair_pred = np.asarray(inputs["pair_pred"][sl]).astype(np.int64)
    bbox = np.asarray(inputs["pair_bbox"][sl], dtype=np.float32)
    uf = np.asarray(inputs["union_features"][sl], dtype=np.float32)
    m = {
        "hidx": _wrap_idx(pair_idx[:, 0]),
        "tidx": _wrap_idx(pair_idx[:, 1]),
        "p0w": _wrap_idx(pair_pred[:, 0]),
        "p1w": _wrap_idx(pair_pred[:, 1]),
        "bboxT": np.ascontiguousarray(bbox.T).astype(NPBF16),
        "uT": np.ascontiguousarray(uf.T).astype(NPBF16),
    }
    m.update(common)
    return m


def _prep_common(inputs):
    f32 = lambda k: np.asarray(inputs[k], dtype=np.float32)
    ectx = f32("edge_ctx").astype(NPBF16)

    freqp = np.zeros((NOC * NOC, 128), dtype=np.float32)
    freqp[:, GOFF : GOFF + NRC] = f32("freq_table")
    freqp = freqp.astype(NPBF16)

    wemb = f32("W_post_emb")  # [512, 1024]
    wcat0 = f32("W_post_cat")  # [1024, 4096]
    # fold: ctx_rep @ W_post_cat == [Eh|Et] @ [[Wh@Wcat_top];[Wt@Wcat_bot]]
    wcat = np.concatenate(
        [wemb[:, :H] @ wcat0[:H], wemb[:, H:] @ wcat0[H:]], axis=0
    )  # [1024, 4096]
    wcat_l = np.ascontiguousarray(
        wcat.reshape(KCAT, 128, MO, 128).transpose(1, 0, 2, 3).reshape(128, KCAT * MO * 128)
    ).astype(NPBF16)

    wspt1_l = f32("W_spt1").astype(NPBF16)  # [32, 512]

    wspt2 = f32("W_spt2")  # [512, 4096]
    wspt2_l = np.ascontiguousarray(
        wspt2.reshape(KC, 128, MO, 128).transpose(2, 1, 0, 3).reshape(MO, 128, KC * 128)
    ).astype(NPBF16)

    wcg = np.zeros((P, 128), dtype=np.float32)
    wcg[:, :NRC] = f32("W_ctx")
    wcg[:, GOFF : GOFF + NRC] = f32("W_gate")
    wcg_l = np.ascontiguousarray(
        wcg.reshape(MO, 128, 128).transpose(1, 0, 2).reshape(128, MO * 128)
    ).astype(NPBF16)

    wvis = np.zeros((P, 128), dtype=np.float32)
    wvis[:, GOFF : GOFF + NRC] = f32("W_vis")
    wvis_l = np.ascontiguousarray(
        wvis.reshape(MO, 128, 128).transpose(1, 0, 2).reshape(128, MO * 128)
    ).astype(NPBF16)

    col = lambda b, n: np.ascontiguousarray(
        np.asarray(b, dtype=np.float32).reshape(n, 128).T
    )
    bctx_l = np.zeros((128, 1), dtype=np.float32)
    bctx_l[:NRC, 0] = f32("b_ctx")
    bvg_l = np.zeros((128, 1), dtype=np.float32)
    bvg_l[GOFF : GOFF + NRC, 0] = f32("b_vis") + f32("b_gate")

    return {
        "ectx": ectx,
        "freqp": freqp,
        "wcat": wcat_l,
        "wspt1": wspt1_l,
        "wspt2": wspt2_l,
        "wcg": wcg_l,
        "wvisp": wvis_l,
        "bcat": col(f32("b_post_emb")[:H] @ wcat0[:H] + f32("b_post_emb")[H:] @ wcat0[H:] + f32("b_post_cat"), MO),
        "bs1": col(inputs["b_spt1"], KC),
        "bs2": col(inputs["b_spt2"], MO),
        "bctx": bctx_l,
        "bvg": bvg_l,
    }


def kernel(**inputs) -> np.ndarray:
    global last_exec_time_ns
    trace = bool(os.environ.get("BASS_KERNEL_TRACE"))
    if trace:
        _register_ntff_hook()
    nc = _build()
    common = _prep_common(inputs)
    in_maps = [_prep_core(inputs, c, common) for c in range(NCORES)]
    res = run_bass_kernel_spmd(nc, in_maps, list(range(NCORES)), trace=trace)
    if trace:
        last_exec_time_ns = res.exec_time_ns
    out = np.concatenate(
        [np.asarray(res.results[c]["out_t"]).T for c in range(NCORES)], axis=0
    )
    return np.ascontiguousarray(out.astype(np.float32))


# revision 12
# speedup vs baseline: 1.0396x; 1.0396x over previous
"""Trainium2 Bass kernel for CausalAnalysisPredictor (gnn_message_passing).

kernel(**inputs) takes the FULL unsharded inputs and returns the FULL
[16384, 51] float32 output. Internally it shards the relation axis across
8 NeuronCores (data-parallel; small weights replicated; per-relation rows
of edge_ctx are gathered on-device via indirect DMA from the replicated
node table).
"""

import os
import sys
import types

import numpy as np

try:
    import concourse  # noqa: F401
except ImportError:  # pragma: no cover
    sys.path.insert(0, "/opt/trn_rl_repo")

import ml_dtypes

import concourse.bass as bass
import concourse.mybir as mybir
import concourse.tile as tile
from concourse import bacc
from concourse.bass import IndirectOffsetOnAxis
from concourse.bass_utils import run_bass_kernel_spmd
from concourse.masks import make_identity

BF16 = mybir.dt.bfloat16
F32 = mybir.dt.float32
I32 = mybir.dt.int32
NPBF16 = ml_dtypes.bfloat16

N_OBJ, N_REL = 4096, 16384
H, P = 512, 4096
NOC, NRC = 151, 51
NCORES = 8
NRELC = N_REL // NCORES  # 2048 relations per core
KC = H // 128            # 4 feat chunks of edge_ctx
KCAT = (2 * H) // 128    # 8 feat chunks of ctx_rep
MO = P // 128            # 32 output-feature chunks
NCH = NRELC // 512       # 4 relation chunks of 512
NG = NRELC // 128        # 16 gather calls per index list
GOFF = 64                # partition offset of the gate/vis/freq lane block

AF = mybir.ActivationFunctionType
ALU = mybir.AluOpType

last_exec_time_ns = None  # set when BASS_KERNEL_TRACE=1


def _register_ntff_hook():
    if "antenv.axon_hooks" in sys.modules:
        return
    hook = None
    try:
        from trn_agent_boot.trn_boot import _ntff_profile_via_ctypes

        hook = _ntff_profile_via_ctypes("/opt/axon/libaxon_pjrt.so")
    except Exception:
        hook = None
    mod = types.ModuleType("antenv.axon_hooks")
    mod.get_axon_ntff_profile_hook = lambda: hook
    mod.set_axon_ntff_profile_hook = lambda h: None
    sys.modules["antenv.axon_hooks"] = mod


_nc_cache = None


def _build():
    global _nc_cache
    if _nc_cache is not None:
        return _nc_cache

    nc = bacc.Bacc("TRN2", target_bir_lowering=False, debug=False, num_devices=NCORES)

    # ---- DRAM parameters (per-core shards / replicated tables) ----
    ectx = nc.declare_dram_parameter("ectx", [N_OBJ, H], BF16, isOutput=False)
    freqp = nc.declare_dram_parameter("freqp", [NOC * NOC, 128], BF16, isOutput=False)
    hidx = nc.declare_dram_parameter("hidx", [128, NG], I32, isOutput=False)
    tidx = nc.declare_dram_parameter("tidx", [128, NG], I32, isOutput=False)
    p0w = nc.declare_dram_parameter("p0w", [128, NG], I32, isOutput=False)
    p1w = nc.declare_dram_parameter("p1w", [128, NG], I32, isOutput=False)
    bboxT = nc.declare_dram_parameter("bboxT", [32, NRELC], BF16, isOutput=False)
    uT = nc.declare_dram_parameter("uT", [P, NRELC], BF16, isOutput=False)
    wcat = nc.declare_dram_parameter("wcat", [MO, 128, KCAT * 128], BF16, isOutput=False)
    wspt1 = nc.declare_dram_parameter("wspt1", [32, H], BF16, isOutput=False)
    wspt2 = nc.declare_dram_parameter("wspt2", [MO, 128, KC * 128], BF16, isOutput=False)
    wcg = nc.declare_dram_parameter("wcg", [128, MO * 128], BF16, isOutput=False)
    wvisp = nc.declare_dram_parameter("wvisp", [128, MO * 128], BF16, isOutput=False)
    bcat = nc.declare_dram_parameter("bcat", [128, MO], F32, isOutput=False)
    bs1 = nc.declare_dram_parameter("bs1", [128, KC], F32, isOutput=False)
    bs2 = nc.declare_dram_parameter("bs2", [128, MO], F32, isOutput=False)
    bctx = nc.declare_dram_parameter("bctx", [128, 1], F32, isOutput=False)
    bvg = nc.declare_dram_parameter("bvg", [128, 1], F32, isOutput=False)
    out_t = nc.declare_dram_parameter("out_t", [NRC, NRELC], F32, isOutput=True)

    with tile.TileContext(nc) as tc:
        with (
            tc.tile_pool(name="sbuf", bufs=1) as pool,
            tc.tile_pool(name="psum", bufs=1, space="PSUM") as pp,
        ):
            # ---- resident small tensors ----
            ident = pool.tile([128, 128], BF16)
            make_identity(nc, ident[:])
            hidx_t = pool.tile([128, NG], I32)
            nc.sync.dma_start(hidx_t[:], hidx[:])
            tidx_t = pool.tile([128, NG], I32)
            nc.sync.dma_start(tidx_t[:], tidx[:])
            p0_t = pool.tile([128, NG], I32)
            nc.sync.dma_start(p0_t[:], p0w[:])
            p1_t = pool.tile([128, NG], I32)
            nc.sync.dma_start(p1_t[:], p1w[:])
            bboxT_t = pool.tile([32, NRELC], BF16)
            nc.sync.dma_start(bboxT_t[:], bboxT[:])
            wspt1_t = pool.tile([32, H], BF16)
            nc.sync.dma_start(wspt1_t[:], wspt1[:])
            wcg_t = pool.tile([128, MO, 128], BF16)
            nc.scalar.dma_start(wcg_t[:], wcg[:].rearrange("p (m c) -> p m c", m=MO))
            wvis_t = pool.tile([128, MO, 128], BF16)
            nc.scalar.dma_start(wvis_t[:], wvisp[:].rearrange("p (m c) -> p m c", m=MO))
            bcat_t = pool.tile([128, MO], F32)
            nc.sync.dma_start(bcat_t[:], bcat[:])
            bs1_t = pool.tile([128, KC], F32)
            nc.sync.dma_start(bs1_t[:], bs1[:])
            bs2_t = pool.tile([128, MO], F32)
            nc.sync.dma_start(bs2_t[:], bs2[:])
            bctx_t = pool.tile([128, 1], F32)
            nc.sync.dma_start(bctx_t[:], bctx[:])
            bvg_t = pool.tile([128, 1], F32)
            nc.sync.dma_start(bvg_t[:], bvg[:])

            # fidx = p0*151 + p1 (int32 on DVE)
            fidx_t = pool.tile([128, NG], I32)
            nc.vector.tensor_scalar(
                out=fidx_t[:], in0=p0_t[:], scalar1=NOC, scalar2=None, op0=ALU.mult
            )
            nc.vector.tensor_add(out=fidx_t[:], in0=fidx_t[:], in1=p1_t[:])

            # ---- gathered + transposed activations ----
            # eT[j]: feature-major gathered edge_ctx; j<KC head chunks, j>=KC tail
            eT = [pool.tile([128, NRELC], BF16, tag=f"eT{j}", name=f"eT{j}") for j in range(2 * KC)]
            gfT = pool.tile([128, NRELC], F32)

            # ---- spt1 (bbox only; PE warm-up during gather prologue) ----
            s1T = [pool.tile([128, NRELC], BF16, tag=f"s1T{k}", name=f"s1T{k}") for k in range(KC)]
            for k in range(KC):
                for n in range(NCH):
                    ps = pp.tile([128, 512], F32, tag="work", bufs=2)
                    nc.tensor.matmul(
                        ps[:],
                        wspt1_t[:, k * 128 : (k + 1) * 128],
                        bboxT_t[:, n * 512 : (n + 1) * 512],
                        start=True,
                        stop=True,
                    )
                    nc.scalar.activation(
                        s1T[k][:, n * 512 : (n + 1) * 512],
                        ps[:],
                        AF.Relu,
                        bias=bs1_t[:, k : k + 1],
                    )

            outT = pool.tile([128, NRELC], F32)
            gs = pool.tile([128, NRELC], BF16)

            def gather_block(idx_tile, src_dram, i, base_j, copy_eng):
                """Gather 128 rows (call i) and PE-transpose into eT[base_j+k] cols."""
                g = pool.tile([128, H], BF16, tag="g", bufs=6)
                nc.gpsimd.indirect_dma_start(
                    out=g[:],
                    out_offset=None,
                    in_=src_dram[:],
                    in_offset=IndirectOffsetOnAxis(ap=idx_tile[:, i : i + 1], axis=0),
                )
                for k in range(KC):
                    pt = pp.tile([128, 128], BF16, tag="work", bufs=2)
                    nc.tensor.transpose(pt[:], g[:, k * 128 : (k + 1) * 128], ident[:])
                    if copy_eng == "act":
                        nc.scalar.activation(
                            eT[base_j + k][:, i * 128 : (i + 1) * 128], pt[:], AF.Copy
                        )
                    else:
                        nc.vector.tensor_copy(
                            out=eT[base_j + k][:, i * 128 : (i + 1) * 128], in_=pt[:]
                        )

            for n in range(NCH):
                nsl = slice(n * 512, (n + 1) * 512)
                # -- gathers for this relation chunk (SWDGE queue runs ahead) --
                for i in range(4 * n, 4 * n + 4):
                    gather_block(hidx_t, ectx, i, 0, "act")
                    gather_block(tidx_t, ectx, i, KC, "dve")
                for i in range(4 * n, 4 * n + 4):
                    gf = pool.tile([128, 128], BF16, tag="gf", bufs=4)
                    nc.gpsimd.indirect_dma_start(
                        out=gf[:],
                        out_offset=None,
                        in_=freqp[:],
                        in_offset=IndirectOffsetOnAxis(ap=fidx_t[:, i : i + 1], axis=0),
                    )
                    ptf = pp.tile([128, 128], BF16, tag="work", bufs=2)
                    nc.tensor.transpose(ptf[:], gf[:], ident[:])
                    nc.scalar.activation(gfT[:, i * 128 : (i + 1) * 128], ptf[:], AF.Copy)

                # -- main: post_cat x spt gate -> ctx/gate/vis heads --
                psum_cg = pp.tile([128, 512], F32, tag="cg", name=f"cg{n}", bufs=2)
                lag = []  # (pc, u_b, m) awaiting their cg/vis matmuls
                for m in range(MO + 1):
                    if m < MO:
                        wcat_b = pool.tile([128, KCAT * 128], BF16, tag="wcat_b", bufs=4)
                        nc.sync.dma_start(wcat_b[:], wcat[m])
                        wspt2_b = pool.tile([128, KC * 128], BF16, tag="wspt2_b", bufs=4)
                        nc.scalar.dma_start(wspt2_b[:], wspt2[m])
                        u_b = pool.tile([128, 512], BF16, tag="u_b", bufs=4)
                        nc.gpsimd.dma_start(u_b[:], uT[m * 128 : (m + 1) * 128, nsl])
                        ps_cat = pp.tile([128, 512], F32, tag="cat", bufs=2)
                        for k in range(KCAT):
                            nc.tensor.matmul(
                                ps_cat[:],
                                wcat_b[:, k * 128 : (k + 1) * 128],
                                eT[k][:, nsl],
                                start=(k == 0),
                                stop=(k == KCAT - 1),
                            )
                        ps_spt = pp.tile([128, 512], F32, tag="spt", bufs=2)
                        for k in range(KC):
                            nc.tensor.matmul(
                                ps_spt[:],
                                wspt2_b[:, k * 128 : (k + 1) * 128],
                                s1T[k][:, nsl],
                                start=(k == 0),
                                stop=(k == KC - 1),
                            )
                        r1 = pool.tile([128, 512], BF16, tag="r1", bufs=3)
                        nc.scalar.activation(
                            r1[:], ps_cat[:], AF.Relu, bias=bcat_t[:, m : m + 1]
                        )
                        r2 = pool.tile([128, 512], BF16, tag="r2", bufs=3)
                        nc.vector.tensor_scalar(
                            out=r2[:],
                            in0=ps_spt[:],
                            scalar1=bs2_t[:, m : m + 1],
                            scalar2=0.0,
                            op0=ALU.add,
                            op1=ALU.max,
                        )
                        pc = pool.tile([128, 512], BF16, tag="pc", bufs=3)
                        nc.vector.tensor_mul(out=pc[:], in0=r1[:], in1=r2[:])
                        lag.append((pc, u_b, m))
                    if lag and (len(lag) > 1 or m == MO):
                        pc_l, u_l, m_l = lag.pop(0)
                        nc.tensor.matmul(
                            psum_cg[:],
                            wcg_t[:, m_l, :],
                            pc_l[:],
                            start=(m_l == 0),
                            stop=False,
                            skip_group_check=True,
                        )
                        nc.tensor.matmul(
                            psum_cg[:],
                            wvis_t[:, m_l, :],
                            u_l[:],
                            start=False,
                            stop=(m_l == MO - 1),
                            skip_group_check=True,
                        )

                # -- epilogue: rel^T = (ctx + b_ctx) * sigmoid(vis+gate+frq+b_vg) --
                sarg = pool.tile([128, 512], F32, tag="sarg", bufs=2)
                nc.vector.tensor_add(
                    out=sarg[GOFF : GOFF + NRC, :],
                    in0=psum_cg[GOFF : GOFF + NRC, :],
                    in1=gfT[GOFF : GOFF + NRC, nsl],
                )
                sg = pool.tile([128, 512], BF16, tag="sg", bufs=2)
                nc.scalar.activation(
                    sg[GOFF : GOFF + NRC, :],
                    sarg[GOFF : GOFF + NRC, :],
                    AF.Sigmoid,
                    bias=bvg_t[GOFF : GOFF + NRC, :],
                )
                # shift sigmoid output from partitions GOFF.. to 0..
                nc.sync.dma_start(gs[0:NRC, nsl], sg[GOFF : GOFF + NRC, :])
                nc.vector.scalar_tensor_tensor(
                    out=outT[0:NRC, nsl],
                    in0=psum_cg[0:NRC, :],
                    scalar=bctx_t[0:NRC, :],
                    in1=gs[0:NRC, nsl],
                    op0=ALU.add,
                    op1=ALU.mult,
                )

            nc.sync.dma_start(out_t[:], outT[0:NRC, :])

    nc.compile()
    _nc_cache = nc
    return _nc_cache


def _wrap_idx(idx):
    """[NRELC] -> [128, NG] int32 with idx[i*128+p] at [p, i]."""
    return np.ascontiguousarray(idx.reshape(NG, 128).T.astype(np.int32))


def _prep_core(inputs, c, common):
    sl = slice(c * NRELC, (c + 1) * NRELC)
    pair_idx = np.asarray(inputs["pair_idx"][sl]).astype(np.int64)
    pair_pred = np.asarray(inputs["pair_pred"][sl]).astype(np.int64)
    bbox = np.asarray(inputs["pair_bbox"][sl], dtype=np.float32)
    uf = np.asarray(inputs["union_features"][sl], dtype=np.float32)
    m = {
        "hidx": _wrap_idx(pair_idx[:, 0]),
        "tidx": _wrap_idx(pair_idx[:, 1]),
        "p0w": _wrap_idx(pair_pred[:, 0]),
        "p1w": _wrap_idx(pair_pred[:, 1]),
        "bboxT": np.ascontiguousarray(bbox.T).astype(NPBF16),
        "uT": np.ascontiguousarray(uf.T).astype(NPBF16),
    }
    m.update(common)
    return m


def _prep_common(inputs):
    f32 = lambda k: np.asarray(inputs[k], dtype=np.float32)
    ectx = f32("edge_ctx").astype(NPBF16)

    freqp = np.zeros((NOC * NOC, 128), dtype=np.float32)
    freqp[:, GOFF : GOFF + NRC] = f32("freq_table")
    freqp = freqp.astype(NPBF16)

    wemb = f32("W_post_emb")  # [512, 1024]
    wcat0 = f32("W_post_cat")  # [1024, 4096]
    # fold: ctx_rep @ W_post_cat == [Eh|Et] @ [[Wh@Wcat_top];[Wt@Wcat_bot]]
    wcat = np.concatenate(
        [wemb[:, :H] @ wcat0[:H], wemb[:, H:] @ wcat0[H:]], axis=0
    )  # [1024, 4096]
    wcat_l = np.ascontiguousarray(
        wcat.reshape(KCAT, 128, MO, 128).transpose(2, 1, 0, 3).reshape(MO, 128, KCAT * 128)
    ).astype(NPBF16)

    wspt1_l = f32("W_spt1").astype(NPBF16)  # [32, 512]

    wspt2 = f32("W_spt2")  # [512, 4096]
    wspt2_l = np.ascontiguousarray(
        wspt2.reshape(KC, 128, MO, 128).transpose(2, 1, 0, 3).reshape(MO, 128, KC * 128)
    ).astype(NPBF16)

    wcg = np.zeros((P, 128), dtype=np.float32)
    wcg[:, :NRC] = f32("W_ctx")
    wcg[:, GOFF : GOFF + NRC] = f32("W_gate")
    wcg_l = np.ascontiguousarray(
        wcg.reshape(MO, 128, 128).transpose(1, 0, 2).reshape(128, MO * 128)
    ).astype(NPBF16)

    wvis = np.zeros((P, 128), dtype=np.float32)
    wvis[:, GOFF : GOFF + NRC] = f32("W_vis")
    wvis_l = np.ascontiguousarray(
        wvis.reshape(MO, 128, 128).transpose(1, 0, 2).reshape(128, MO * 128)
    ).astype(NPBF16)

    col = lambda b, n: np.ascontiguousarray(
        np.asarray(b, dtype=np.float32).reshape(n, 128).T
    )
    bctx_l = np.zeros((128, 1), dtype=np.float32)
    bctx_l[:NRC, 0] = f32("b_ctx")
    bvg_l = np.zeros((128, 1), dtype=np.float32)
    bvg_l[GOFF : GOFF + NRC, 0] = f32("b_vis") + f32("b_gate")

    return {
        "ectx": ectx,
        "freqp": freqp,
        "wcat": wcat_l,
        "wspt1": wspt1_l,
        "wspt2": wspt2_l,
        "wcg": wcg_l,
        "wvisp": wvis_l,
        "bcat": col(f32("b_post_emb")[:H] @ wcat0[:H] + f32("b_post_emb")[H:] @ wcat0[H:] + f32("b_post_cat"), MO),
        "bs1": col(inputs["b_spt1"], KC),
        "bs2": col(inputs["b_spt2"], MO),
        "bctx": bctx_l,
        "bvg": bvg_l,
    }


def kernel(**inputs) -> np.ndarray:
    global last_exec_time_ns
    trace = bool(os.environ.get("BASS_KERNEL_TRACE"))
    if trace:
        _register_ntff_hook()
    nc = _build()
    common = _prep_common(inputs)
    in_maps = [_prep_core(inputs, c, common) for c in range(NCORES)]
    res = run_bass_kernel_spmd(nc, in_maps, list(range(NCORES)), trace=trace)
    if trace:
        last_exec_time_ns = res.exec_time_ns
    out = np.concatenate(
        [np.asarray(res.results[c]["out_t"]).T for c in range(NCORES)], axis=0
    )
    return np.ascontiguousarray(out.astype(np.float32))


# revision 13
# speedup vs baseline: 1.0560x; 1.0157x over previous
"""Trainium2 Bass kernel for CausalAnalysisPredictor (gnn_message_passing).

kernel(**inputs) takes the FULL unsharded inputs and returns the FULL
[16384, 51] float32 output. Internally it shards the relation axis across
8 NeuronCores (data-parallel; small weights replicated; per-relation rows
of edge_ctx are gathered on-device via indirect DMA from the replicated
node table).
"""

import os
import sys
import types

import numpy as np

try:
    import concourse  # noqa: F401
except ImportError:  # pragma: no cover
    sys.path.insert(0, "/opt/trn_rl_repo")

import ml_dtypes

import concourse.bass as bass
import concourse.mybir as mybir
import concourse.tile as tile
from concourse import bacc
from concourse.bass import IndirectOffsetOnAxis
from concourse.bass_utils import run_bass_kernel_spmd
from concourse.masks import make_identity

BF16 = mybir.dt.bfloat16
F32 = mybir.dt.float32
I32 = mybir.dt.int32
NPBF16 = ml_dtypes.bfloat16

N_OBJ, N_REL = 4096, 16384
H, P = 512, 4096
NOC, NRC = 151, 51
NCORES = 8
NRELC = N_REL // NCORES  # 2048 relations per core
KC = H // 128            # 4 feat chunks of edge_ctx
KCAT = (2 * H) // 128    # 8 feat chunks of ctx_rep
MO = P // 128            # 32 output-feature chunks
NCH = NRELC // 512       # 4 relation chunks of 512
NG = NRELC // 128        # 16 gather calls per index list
GOFF = 64                # partition offset of the gate/vis/freq lane block

AF = mybir.ActivationFunctionType
ALU = mybir.AluOpType

last_exec_time_ns = None  # set when BASS_KERNEL_TRACE=1


def _register_ntff_hook():
    if "antenv.axon_hooks" in sys.modules:
        return
    hook = None
    try:
        from trn_agent_boot.trn_boot import _ntff_profile_via_ctypes

        hook = _ntff_profile_via_ctypes("/opt/axon/libaxon_pjrt.so")
    except Exception:
        hook = None
    mod = types.ModuleType("antenv.axon_hooks")
    mod.get_axon_ntff_profile_hook = lambda: hook
    mod.set_axon_ntff_profile_hook = lambda h: None
    sys.modules["antenv.axon_hooks"] = mod


_nc_cache = None


def _build():
    global _nc_cache
    if _nc_cache is not None:
        return _nc_cache

    nc = bacc.Bacc("TRN2", target_bir_lowering=False, debug=False, num_devices=NCORES)

    # ---- DRAM parameters (per-core shards / replicated tables) ----
    ectx = nc.declare_dram_parameter("ectx", [N_OBJ, H], BF16, isOutput=False)
    freqp = nc.declare_dram_parameter("freqp", [NOC * NOC, 128], BF16, isOutput=False)
    hidx = nc.declare_dram_parameter("hidx", [128, NG], I32, isOutput=False)
    tidx = nc.declare_dram_parameter("tidx", [128, NG], I32, isOutput=False)
    p0w = nc.declare_dram_parameter("p0w", [128, NG], I32, isOutput=False)
    p1w = nc.declare_dram_parameter("p1w", [128, NG], I32, isOutput=False)
    bboxT = nc.declare_dram_parameter("bboxT", [32, NRELC], BF16, isOutput=False)
    uT = nc.declare_dram_parameter("uT", [P, NRELC], BF16, isOutput=False)
    wcat = nc.declare_dram_parameter("wcat", [MO, 128, KCAT * 128], BF16, isOutput=False)
    wspt1 = nc.declare_dram_parameter("wspt1", [32, H], BF16, isOutput=False)
    wspt2 = nc.declare_dram_parameter("wspt2", [MO, 128, KC * 128], BF16, isOutput=False)
    wcg = nc.declare_dram_parameter("wcg", [128, MO * 128], BF16, isOutput=False)
    wvisp = nc.declare_dram_parameter("wvisp", [128, MO * 128], BF16, isOutput=False)
    bcat = nc.declare_dram_parameter("bcat", [128, MO], F32, isOutput=False)
    bs1 = nc.declare_dram_parameter("bs1", [128, KC], F32, isOutput=False)
    bs2 = nc.declare_dram_parameter("bs2", [128, MO], F32, isOutput=False)
    bctx = nc.declare_dram_parameter("bctx", [128, 1], F32, isOutput=False)
    bvg = nc.declare_dram_parameter("bvg", [128, 1], F32, isOutput=False)
    out_t = nc.declare_dram_parameter("out_t", [NRC, NRELC], F32, isOutput=True)

    with tile.TileContext(nc) as tc:
        with (
            tc.tile_pool(name="sbuf", bufs=1) as pool,
            tc.tile_pool(name="psum", bufs=1, space="PSUM") as pp,
        ):
            # ---- resident small tensors ----
            ident = pool.tile([128, 128], BF16)
            make_identity(nc, ident[:])
            hidx_t = pool.tile([128, NG], I32)
            nc.sync.dma_start(hidx_t[:], hidx[:])
            tidx_t = pool.tile([128, NG], I32)
            nc.sync.dma_start(tidx_t[:], tidx[:])
            p0_t = pool.tile([128, NG], I32)
            nc.sync.dma_start(p0_t[:], p0w[:])
            p1_t = pool.tile([128, NG], I32)
            nc.sync.dma_start(p1_t[:], p1w[:])
            bboxT_t = pool.tile([32, NRELC], BF16)
            nc.sync.dma_start(bboxT_t[:], bboxT[:])
            wspt1_t = pool.tile([32, H], BF16)
            nc.sync.dma_start(wspt1_t[:], wspt1[:])
            wcg_t = pool.tile([128, MO, 128], BF16)
            nc.scalar.dma_start(wcg_t[:], wcg[:].rearrange("p (m c) -> p m c", m=MO))
            wvis_t = pool.tile([128, MO, 128], BF16)
            nc.scalar.dma_start(wvis_t[:], wvisp[:].rearrange("p (m c) -> p m c", m=MO))
            bcat_t = pool.tile([128, MO], F32)
            nc.sync.dma_start(bcat_t[:], bcat[:])
            bs1_t = pool.tile([128, KC], F32)
            nc.sync.dma_start(bs1_t[:], bs1[:])
            bs2_t = pool.tile([128, MO], F32)
            nc.sync.dma_start(bs2_t[:], bs2[:])
            bctx_t = pool.tile([128, 1], F32)
            nc.sync.dma_start(bctx_t[:], bctx[:])
            bvg_t = pool.tile([128, 1], F32)
            nc.sync.dma_start(bvg_t[:], bvg[:])

            # fidx = p0*151 + p1 (int32 on DVE)
            fidx_t = pool.tile([128, NG], I32)
            nc.vector.tensor_scalar(
                out=fidx_t[:], in0=p0_t[:], scalar1=NOC, scalar2=None, op0=ALU.mult
            )
            nc.vector.tensor_add(out=fidx_t[:], in0=fidx_t[:], in1=p1_t[:])

            # ---- gathered + transposed activations ----
            # eT[j]: feature-major gathered edge_ctx; j<KC head chunks, j>=KC tail
            eT = [pool.tile([128, NRELC], BF16, tag=f"eT{j}", name=f"eT{j}") for j in range(2 * KC)]
            gfT = pool.tile([128, NRELC], F32)

            # ---- spt1 (bbox only; PE warm-up during gather prologue) ----
            s1T = [pool.tile([128, NRELC], BF16, tag=f"s1T{k}", name=f"s1T{k}") for k in range(KC)]
            for k in range(KC):
                for n in range(NCH):
                    ps = pp.tile([128, 512], F32, tag="work", bufs=2)
                    nc.tensor.matmul(
                        ps[:],
                        wspt1_t[:, k * 128 : (k + 1) * 128],
                        bboxT_t[:, n * 512 : (n + 1) * 512],
                        start=True,
                        stop=True,
                    )
                    nc.scalar.activation(
                        s1T[k][:, n * 512 : (n + 1) * 512],
                        ps[:],
                        AF.Relu,
                        bias=bs1_t[:, k : k + 1],
                    )

            outT = pool.tile([128, NRELC], F32)
            gs = pool.tile([128, NRELC], BF16)

            def gather_block(idx_tile, src_dram, i, base_j, copy_eng):
                """Gather 128 rows (call i) and PE-transpose into eT[base_j+k] cols."""
                g = pool.tile([128, H], BF16, tag="g", bufs=6)
                nc.gpsimd.indirect_dma_start(
                    out=g[:],
                    out_offset=None,
                    in_=src_dram[:],
                    in_offset=IndirectOffsetOnAxis(ap=idx_tile[:, i : i + 1], axis=0),
                )
                for k in range(KC):
                    pt = pp.tile([128, 128], BF16, tag="work", bufs=2)
                    nc.tensor.transpose(pt[:], g[:, k * 128 : (k + 1) * 128], ident[:])
                    if copy_eng == "act":
                        nc.scalar.activation(
                            eT[base_j + k][:, i * 128 : (i + 1) * 128], pt[:], AF.Copy
                        )
                    else:
                        nc.vector.tensor_copy(
                            out=eT[base_j + k][:, i * 128 : (i + 1) * 128], in_=pt[:]
                        )

            for n in range(NCH):
                nsl = slice(n * 512, (n + 1) * 512)
                # -- gathers for this relation chunk (SWDGE queue runs ahead) --
                for i in range(4 * n, 4 * n + 4):
                    gather_block(hidx_t, ectx, i, 0, "act")
                    gather_block(tidx_t, ectx, i, KC, "dve")
                for i in range(4 * n, 4 * n + 4):
                    gf = pool.tile([128, 128], BF16, tag="gf", bufs=4)
                    nc.gpsimd.indirect_dma_start(
                        out=gf[:],
                        out_offset=None,
                        in_=freqp[:],
                        in_offset=IndirectOffsetOnAxis(ap=fidx_t[:, i : i + 1], axis=0),
                    )
                    ptf = pp.tile([128, 128], BF16, tag="work", bufs=2)
                    nc.tensor.transpose(ptf[:], gf[:], ident[:])
                    nc.scalar.activation(gfT[:, i * 128 : (i + 1) * 128], ptf[:], AF.Copy)

                # -- main: post_cat x spt gate -> ctx/gate/vis heads --
                psum_cg = pp.tile([128, 512], F32, tag="cg", name=f"cg{n}", bufs=2)
                lag = []  # (pc, u_b, m) awaiting their cg/vis matmuls
                for m in range(MO + 1):
                    if m < MO:
                        wcat_b = pool.tile([128, KCAT * 128], BF16, tag="wcat_b", bufs=4)
                        nc.sync.dma_start(wcat_b[:], wcat[m])
                        wspt2_b = pool.tile([128, KC * 128], BF16, tag="wspt2_b", bufs=4)
                        nc.scalar.dma_start(wspt2_b[:], wspt2[m])
                        u_b = pool.tile([128, 512], BF16, tag="u_b", bufs=4)
                        nc.scalar.dma_start(u_b[:], uT[m * 128 : (m + 1) * 128, nsl])
                        ps_cat = pp.tile([128, 512], F32, tag="cat", bufs=2)
                        for k in range(KCAT):
                            nc.tensor.matmul(
                                ps_cat[:],
                                wcat_b[:, k * 128 : (k + 1) * 128],
                                eT[k][:, nsl],
                                start=(k == 0),
                                stop=(k == KCAT - 1),
                            )
                        ps_spt = pp.tile([128, 512], F32, tag="spt", bufs=2)
                        for k in range(KC):
                            nc.tensor.matmul(
                                ps_spt[:],
                                wspt2_b[:, k * 128 : (k + 1) * 128],
                                s1T[k][:, nsl],
                                start=(k == 0),
                                stop=(k == KC - 1),
                            )
                        r1 = pool.tile([128, 512], BF16, tag="r1", bufs=3)
                        nc.scalar.activation(
                            r1[:], ps_cat[:], AF.Relu, bias=bcat_t[:, m : m + 1]
                        )
                        r2 = pool.tile([128, 512], BF16, tag="r2", bufs=3)
                        nc.vector.tensor_scalar(
                            out=r2[:],
                            in0=ps_spt[:],
                            scalar1=bs2_t[:, m : m + 1],
                            scalar2=0.0,
                            op0=ALU.add,
                            op1=ALU.max,
                        )
                        pc = pool.tile([128, 512], BF16, tag="pc", bufs=3)
                        nc.vector.tensor_mul(out=pc[:], in0=r1[:], in1=r2[:])
                        lag.append((pc, u_b, m))
                    if lag and (len(lag) > 1 or m == MO):
                        pc_l, u_l, m_l = lag.pop(0)
                        nc.tensor.matmul(
                            psum_cg[:],
                            wcg_t[:, m_l, :],
                            pc_l[:],
                            start=(m_l == 0),
                            stop=False,
                            skip_group_check=True,
                        )
                        nc.tensor.matmul(
                            psum_cg[:],
                            wvis_t[:, m_l, :],
                            u_l[:],
                            start=False,
                            stop=(m_l == MO - 1),
                            skip_group_check=True,
                        )

                # -- epilogue: rel^T = (ctx + b_ctx) * sigmoid(vis+gate+frq+b_vg) --
                sarg = pool.tile([128, 512], F32, tag="sarg", bufs=2)
                nc.vector.tensor_add(
                    out=sarg[GOFF : GOFF + NRC, :],
                    in0=psum_cg[GOFF : GOFF + NRC, :],
                    in1=gfT[GOFF : GOFF + NRC, nsl],
                )
                sg = pool.tile([128, 512], BF16, tag="sg", bufs=2)
                nc.scalar.activation(
                    sg[GOFF : GOFF + NRC, :],
                    sarg[GOFF : GOFF + NRC, :],
                    AF.Sigmoid,
                    bias=bvg_t[GOFF : GOFF + NRC, :],
                )
                # shift sigmoid output from partitions GOFF.. to 0..
                nc.sync.dma_start(gs[0:NRC, nsl], sg[GOFF : GOFF + NRC, :])
                nc.vector.scalar_tensor_tensor(
                    out=outT[0:NRC, nsl],
                    in0=psum_cg[0:NRC, :],
                    scalar=bctx_t[0:NRC, :],
                    in1=gs[0:NRC, nsl],
                    op0=ALU.add,
                    op1=ALU.mult,
                )
                nc.sync.dma_start(out_t[:, nsl], outT[0:NRC, nsl])

    nc.compile()
    _nc_cache = nc
    return _nc_cache


def _wrap_idx(idx):
    """[NRELC] -> [128, NG] int32 with idx[i*128+p] at [p, i]."""
    return np.ascontiguousarray(idx.reshape(NG, 128).T.astype(np.int32))


def _prep_core(inputs, c, common):
    sl = slice(c * NRELC, (c + 1) * NRELC)
    pair_idx = np.asarray(inputs["pair_idx"][sl]).astype(np.int64)
    pair_pred = np.asarray(inputs["pair_pred"][sl]).astype(np.int64)
    bbox = np.asarray(inputs["pair_bbox"][sl], dtype=np.float32)
    uf = np.asarray(inputs["union_features"][sl], dtype=np.float32)
    m = {
        "hidx": _wrap_idx(pair_idx[:, 0]),
        "tidx": _wrap_idx(pair_idx[:, 1]),
        "p0w": _wrap_idx(pair_pred[:, 0]),
        "p1w": _wrap_idx(pair_pred[:, 1]),
        "bboxT": np.ascontiguousarray(bbox.T).astype(NPBF16),
        "uT": np.ascontiguousarray(uf.T).astype(NPBF16),
    }
    m.update(common)
    return m


def _prep_common(inputs):
    f32 = lambda k: np.asarray(inputs[k], dtype=np.float32)
    ectx = f32("edge_ctx").astype(NPBF16)

    freqp = np.zeros((NOC * NOC, 128), dtype=np.float32)
    freqp[:, GOFF : GOFF + NRC] = f32("freq_table")
    freqp = freqp.astype(NPBF16)

    wemb = f32("W_post_emb")  # [512, 1024]
    wcat0 = f32("W_post_cat")  # [1024, 4096]
    # fold: ctx_rep @ W_post_cat == [Eh|Et] @ [[Wh@Wcat_top];[Wt@Wcat_bot]]
    wcat = np.concatenate(
        [wemb[:, :H] @ wcat0[:H], wemb[:, H:] @ wcat0[H:]], axis=0
    )  # [1024, 4096]
    wcat_l = np.ascontiguousarray(
        wcat.reshape(KCAT, 128, MO, 128).transpose(2, 1, 0, 3).reshape(MO, 128, KCAT * 128)
    ).astype(NPBF16)

    wspt1_l = f32("W_spt1").astype(NPBF16)  # [32, 512]

    wspt2 = f32("W_spt2")  # [512, 4096]
    wspt2_l = np.ascontiguousarray(
        wspt2.reshape(KC, 128, MO, 128).transpose(2, 1, 0, 3).reshape(MO, 128, KC * 128)
    ).astype(NPBF16)

    wcg = np.zeros((P, 128), dtype=np.float32)
    wcg[:, :NRC] = f32("W_ctx")
    wcg[:, GOFF : GOFF + NRC] = f32("W_gate")
    wcg_l = np.ascontiguousarray(
        wcg.reshape(MO, 128, 128).transpose(1, 0, 2).reshape(128, MO * 128)
    ).astype(NPBF16)

    wvis = np.zeros((P, 128), dtype=np.float32)
    wvis[:, GOFF : GOFF + NRC] = f32("W_vis")
    wvis_l = np.ascontiguousarray(
        wvis.reshape(MO, 128, 128).transpose(1, 0, 2).reshape(128, MO * 128)
    ).astype(NPBF16)

    col = lambda b, n: np.ascontiguousarray(
        np.asarray(b, dtype=np.float32).reshape(n, 128).T
    )
    bctx_l = np.zeros((128, 1), dtype=np.float32)
    bctx_l[:NRC, 0] = f32("b_ctx")
    bvg_l = np.zeros((128, 1), dtype=np.float32)
    bvg_l[GOFF : GOFF + NRC, 0] = f32("b_vis") + f32("b_gate")

    return {
        "ectx": ectx,
        "freqp": freqp,
        "wcat": wcat_l,
        "wspt1": wspt1_l,
        "wspt2": wspt2_l,
        "wcg": wcg_l,
        "wvisp": wvis_l,
        "bcat": col(f32("b_post_emb")[:H] @ wcat0[:H] + f32("b_post_emb")[H:] @ wcat0[H:] + f32("b_post_cat"), MO),
        "bs1": col(inputs["b_spt1"], KC),
        "bs2": col(inputs["b_spt2"], MO),
        "bctx": bctx_l,
        "bvg": bvg_l,
    }


def kernel(**inputs) -> np.ndarray:
    global last_exec_time_ns
    trace = bool(os.environ.get("BASS_KERNEL_TRACE"))
    if trace:
        _register_ntff_hook()
    nc = _build()
    common = _prep_common(inputs)
    in_maps = [_prep_core(inputs, c, common) for c in range(NCORES)]
    res = run_bass_kernel_spmd(nc, in_maps, list(range(NCORES)), trace=trace)
    if trace:
        last_exec_time_ns = res.exec_time_ns
    out = np.concatenate(
        [np.asarray(res.results[c]["out_t"]).T for c in range(NCORES)], axis=0
    )
    return np.ascontiguousarray(out.astype(np.float32))


# revision 14
# speedup vs baseline: 1.0580x; 1.0019x over previous
"""Trainium2 Bass kernel for CausalAnalysisPredictor (gnn_message_passing).

kernel(**inputs) takes the FULL unsharded inputs and returns the FULL
[16384, 51] float32 output. Internally it shards the relation axis across
8 NeuronCores (data-parallel; small weights replicated; per-relation rows
of edge_ctx are gathered on-device via indirect DMA from the replicated
node table).
"""

import os
import sys
import types

import numpy as np

try:
    import concourse  # noqa: F401
except ImportError:  # pragma: no cover
    sys.path.insert(0, "/opt/trn_rl_repo")

import ml_dtypes

import concourse.bass as bass
import concourse.mybir as mybir
import concourse.tile as tile
from concourse import bacc
from concourse.bass import IndirectOffsetOnAxis
from concourse.bass_utils import run_bass_kernel_spmd
from concourse.masks import make_identity

BF16 = mybir.dt.bfloat16
F32 = mybir.dt.float32
I32 = mybir.dt.int32
NPBF16 = ml_dtypes.bfloat16

N_OBJ, N_REL = 4096, 16384
H, P = 512, 4096
NOC, NRC = 151, 51
NCORES = 8
NRELC = N_REL // NCORES  # 2048 relations per core
KC = H // 128            # 4 feat chunks of edge_ctx
KCAT = (2 * H) // 128    # 8 feat chunks of ctx_rep
MO = P // 128            # 32 output-feature chunks
NCH = NRELC // 512       # 4 relation chunks of 512
NG = NRELC // 128        # 16 gather calls per index list
GOFF = 64                # partition offset of the gate/vis/freq lane block

AF = mybir.ActivationFunctionType
ALU = mybir.AluOpType

last_exec_time_ns = None  # set when BASS_KERNEL_TRACE=1


def _register_ntff_hook():
    if "antenv.axon_hooks" in sys.modules:
        return
    hook = None
    try:
        from trn_agent_boot.trn_boot import _ntff_profile_via_ctypes

        hook = _ntff_profile_via_ctypes("/opt/axon/libaxon_pjrt.so")
    except Exception:
        hook = None
    mod = types.ModuleType("antenv.axon_hooks")
    mod.get_axon_ntff_profile_hook = lambda: hook
    mod.set_axon_ntff_profile_hook = lambda h: None
    sys.modules["antenv.axon_hooks"] = mod


_nc_cache = None


def _build():
    global _nc_cache
    if _nc_cache is not None:
        return _nc_cache

    nc = bacc.Bacc("TRN2", target_bir_lowering=False, debug=False, num_devices=NCORES)

    # ---- DRAM parameters (per-core shards / replicated tables) ----
    ectx = nc.declare_dram_parameter("ectx", [N_OBJ, H], BF16, isOutput=False)
    freqp = nc.declare_dram_parameter("freqp", [NOC * NOC, 128], BF16, isOutput=False)
    hidx = nc.declare_dram_parameter("hidx", [128, NG], I32, isOutput=False)
    tidx = nc.declare_dram_parameter("tidx", [128, NG], I32, isOutput=False)
    p0w = nc.declare_dram_parameter("p0w", [128, NG], I32, isOutput=False)
    p1w = nc.declare_dram_parameter("p1w", [128, NG], I32, isOutput=False)
    bboxT = nc.declare_dram_parameter("bboxT", [32, NRELC], BF16, isOutput=False)
    uT = nc.declare_dram_parameter("uT", [P, NRELC], BF16, isOutput=False)
    wcat = nc.declare_dram_parameter("wcat", [MO, 128, KCAT * 128], BF16, isOutput=False)
    wspt1 = nc.declare_dram_parameter("wspt1", [32, H], BF16, isOutput=False)
    wspt2 = nc.declare_dram_parameter("wspt2", [MO, 128, KC * 128], BF16, isOutput=False)
    wcg = nc.declare_dram_parameter("wcg", [128, MO * 128], BF16, isOutput=False)
    wvisp = nc.declare_dram_parameter("wvisp", [128, MO * 128], BF16, isOutput=False)
    bcat = nc.declare_dram_parameter("bcat", [128, MO], F32, isOutput=False)
    bs1 = nc.declare_dram_parameter("bs1", [128, KC], F32, isOutput=False)
    bs2 = nc.declare_dram_parameter("bs2", [128, MO], F32, isOutput=False)
    bctx = nc.declare_dram_parameter("bctx", [128, 1], F32, isOutput=False)
    bvg = nc.declare_dram_parameter("bvg", [128, 1], F32, isOutput=False)
    out_t = nc.declare_dram_parameter("out_t", [NRC, NRELC], F32, isOutput=True)

    with tile.TileContext(nc) as tc:
        with (
            tc.tile_pool(name="sbuf", bufs=1) as pool,
            tc.tile_pool(name="psum", bufs=1, space="PSUM") as pp,
        ):
            # ---- resident small tensors ----
            ident = pool.tile([128, 128], BF16)
            make_identity(nc, ident[:])
            hidx_t = pool.tile([128, NG], I32)
            nc.sync.dma_start(hidx_t[:], hidx[:])
            tidx_t = pool.tile([128, NG], I32)
            nc.sync.dma_start(tidx_t[:], tidx[:])
            p0_t = pool.tile([128, NG], I32)
            nc.sync.dma_start(p0_t[:], p0w[:])
            p1_t = pool.tile([128, NG], I32)
            nc.sync.dma_start(p1_t[:], p1w[:])
            wspt1_t = pool.tile([32, H], BF16)
            nc.sync.dma_start(wspt1_t[:], wspt1[:])
            bboxT_t = pool.tile([32, NRELC], BF16)
            nc.sync.dma_start(bboxT_t[:], bboxT[:])
            wcg_t = pool.tile([128, MO, 128], BF16)
            nc.scalar.dma_start(wcg_t[:], wcg[:].rearrange("p (m c) -> p m c", m=MO))
            wvis_t = pool.tile([128, MO, 128], BF16)
            nc.scalar.dma_start(wvis_t[:], wvisp[:].rearrange("p (m c) -> p m c", m=MO))
            bcat_t = pool.tile([128, MO], F32)
            nc.sync.dma_start(bcat_t[:], bcat[:])
            bs1_t = pool.tile([128, KC], F32)
            nc.sync.dma_start(bs1_t[:], bs1[:])
            bs2_t = pool.tile([128, MO], F32)
            nc.sync.dma_start(bs2_t[:], bs2[:])
            bctx_t = pool.tile([128, 1], F32)
            nc.sync.dma_start(bctx_t[:], bctx[:])
            bvg_t = pool.tile([128, 1], F32)
            nc.sync.dma_start(bvg_t[:], bvg[:])

            # fidx = p0*151 + p1 (int32 on DVE)
            fidx_t = pool.tile([128, NG], I32)
            nc.vector.tensor_scalar(
                out=fidx_t[:], in0=p0_t[:], scalar1=NOC, scalar2=None, op0=ALU.mult
            )
            nc.vector.tensor_add(out=fidx_t[:], in0=fidx_t[:], in1=p1_t[:])

            # ---- gathered + transposed activations ----
            # eT[j]: feature-major gathered edge_ctx; j<KC head chunks, j>=KC tail
            eT = [pool.tile([128, NRELC], BF16, tag=f"eT{j}", name=f"eT{j}") for j in range(2 * KC)]
            gfT = pool.tile([128, NRELC], F32)

            # ---- spt1 (bbox only; PE warm-up during gather prologue) ----
            s1T = [pool.tile([128, NRELC], BF16, tag=f"s1T{k}", name=f"s1T{k}") for k in range(KC)]
            for k in range(KC):
                for n in range(NCH):
                    ps = pp.tile([128, 512], F32, tag="work", bufs=2)
                    nc.tensor.matmul(
                        ps[:],
                        wspt1_t[:, k * 128 : (k + 1) * 128],
                        bboxT_t[:, n * 512 : (n + 1) * 512],
                        start=True,
                        stop=True,
                    )
                    nc.scalar.activation(
                        s1T[k][:, n * 512 : (n + 1) * 512],
                        ps[:],
                        AF.Relu,
                        bias=bs1_t[:, k : k + 1],
                    )

            outT = pool.tile([128, NRELC], F32)
            gs = pool.tile([128, NRELC], BF16)

            def gather_block(idx_tile, src_dram, i, base_j, copy_eng):
                """Gather 128 rows (call i) and PE-transpose into eT[base_j+k] cols."""
                g = pool.tile([128, H], BF16, tag="g", bufs=6)
                nc.gpsimd.indirect_dma_start(
                    out=g[:],
                    out_offset=None,
                    in_=src_dram[:],
                    in_offset=IndirectOffsetOnAxis(ap=idx_tile[:, i : i + 1], axis=0),
                )
                for k in range(KC):
                    pt = pp.tile([128, 128], BF16, tag="work", bufs=2)
                    nc.tensor.transpose(pt[:], g[:, k * 128 : (k + 1) * 128], ident[:])
                    if copy_eng == "act":
                        nc.scalar.activation(
                            eT[base_j + k][:, i * 128 : (i + 1) * 128], pt[:], AF.Copy
                        )
                    else:
                        nc.vector.tensor_copy(
                            out=eT[base_j + k][:, i * 128 : (i + 1) * 128], in_=pt[:]
                        )

            for n in range(NCH):
                nsl = slice(n * 512, (n + 1) * 512)
                # -- gathers for this relation chunk (SWDGE queue runs ahead) --
                for i in range(4 * n, 4 * n + 4):
                    gather_block(hidx_t, ectx, i, 0, "act")
                    gather_block(tidx_t, ectx, i, KC, "dve")
                for i in range(4 * n, 4 * n + 4):
                    gf = pool.tile([128, 128], BF16, tag="gf", bufs=4)
                    nc.gpsimd.indirect_dma_start(
                        out=gf[:],
                        out_offset=None,
                        in_=freqp[:],
                        in_offset=IndirectOffsetOnAxis(ap=fidx_t[:, i : i + 1], axis=0),
                    )
                    ptf = pp.tile([128, 128], BF16, tag="work", bufs=2)
                    nc.tensor.transpose(ptf[:], gf[:], ident[:])
                    nc.scalar.activation(gfT[:, i * 128 : (i + 1) * 128], ptf[:], AF.Copy)

                # -- main: post_cat x spt gate -> ctx/gate/vis heads --
                psum_cg = pp.tile([128, 512], F32, tag="cg", name=f"cg{n}", bufs=2)
                lag = []  # (pc, u_b, m) awaiting their cg/vis matmuls
                for m in range(MO + 1):
                    if m < MO:
                        wcat_b = pool.tile([128, KCAT * 128], BF16, tag="wcat_b", bufs=4)
                        nc.sync.dma_start(wcat_b[:], wcat[m])
                        wspt2_b = pool.tile([128, KC * 128], BF16, tag="wspt2_b", bufs=4)
                        nc.scalar.dma_start(wspt2_b[:], wspt2[m])
                        u_b = pool.tile([128, 512], BF16, tag="u_b", bufs=4)
                        nc.scalar.dma_start(u_b[:], uT[m * 128 : (m + 1) * 128, nsl])
                        ps_cat = pp.tile([128, 512], F32, tag="cat", bufs=2)
                        for k in range(KCAT):
                            nc.tensor.matmul(
                                ps_cat[:],
                                wcat_b[:, k * 128 : (k + 1) * 128],
                                eT[k][:, nsl],
                                start=(k == 0),
                                stop=(k == KCAT - 1),
                            )
                        ps_spt = pp.tile([128, 512], F32, tag="spt", bufs=2)
                        for k in range(KC):
                            nc.tensor.matmul(
                                ps_spt[:],
                                wspt2_b[:, k * 128 : (k + 1) * 128],
                                s1T[k][:, nsl],
                                start=(k == 0),
                                stop=(k == KC - 1),
                            )
                        r1 = pool.tile([128, 512], BF16, tag="r1", bufs=3)
                        nc.scalar.activation(
                            r1[:], ps_cat[:], AF.Relu, bias=bcat_t[:, m : m + 1]
                        )
                        r2 = pool.tile([128, 512], BF16, tag="r2", bufs=3)
                        nc.vector.tensor_scalar(
                            out=r2[:],
                            in0=ps_spt[:],
                            scalar1=bs2_t[:, m : m + 1],
                            scalar2=0.0,
                            op0=ALU.add,
                            op1=ALU.max,
                        )
                        pc = pool.tile([128, 512], BF16, tag="pc", bufs=3)
                        nc.vector.tensor_mul(out=pc[:], in0=r1[:], in1=r2[:])
                        lag.append((pc, u_b, m))
                    if lag and (len(lag) > 1 or m == MO):
                        pc_l, u_l, m_l = lag.pop(0)
                        nc.tensor.matmul(
                            psum_cg[:],
                            wcg_t[:, m_l, :],
                            pc_l[:],
                            start=(m_l == 0),
                            stop=False,
                            skip_group_check=True,
                        )
                        nc.tensor.matmul(
                            psum_cg[:],
                            wvis_t[:, m_l, :],
                            u_l[:],
                            start=False,
                            stop=(m_l == MO - 1),
                            skip_group_check=True,
                        )

                # -- epilogue: rel^T = (ctx + b_ctx) * sigmoid(vis+gate+frq+b_vg) --
                sarg = pool.tile([128, 512], F32, tag="sarg", bufs=2)
                nc.vector.tensor_add(
                    out=sarg[GOFF : GOFF + NRC, :],
                    in0=psum_cg[GOFF : GOFF + NRC, :],
                    in1=gfT[GOFF : GOFF + NRC, nsl],
                )
                sg = pool.tile([128, 512], BF16, tag="sg", bufs=2)
                nc.scalar.activation(
                    sg[GOFF : GOFF + NRC, :],
                    sarg[GOFF : GOFF + NRC, :],
                    AF.Sigmoid,
                    bias=bvg_t[GOFF : GOFF + NRC, :],
                )
                # shift sigmoid output from partitions GOFF.. to 0..
                nc.sync.dma_start(gs[0:NRC, nsl], sg[GOFF : GOFF + NRC, :])
                nc.vector.scalar_tensor_tensor(
                    out=outT[0:NRC, nsl],
                    in0=psum_cg[0:NRC, :],
                    scalar=bctx_t[0:NRC, :],
                    in1=gs[0:NRC, nsl],
                    op0=ALU.add,
                    op1=ALU.mult,
                )
                nc.sync.dma_start(out_t[:, nsl], outT[0:NRC, nsl])

    nc.compile()
    _nc_cache = nc
    return _nc_cache


def _wrap_idx(idx):
    """[NRELC] -> [128, NG] int32 with idx[i*128+p] at [p, i]."""
    return np.ascontiguousarray(idx.reshape(NG, 128).T.astype(np.int32))


def _prep_core(inputs, c, common):
    sl = slice(c * NRELC, (c + 1) * NRELC)
    pair_idx = np.asarray(inputs["pair_idx"][sl]).astype(np.int64)
    pair_pred = np.asarray(inputs["pair_pred"][sl]).astype(np.int64)
    bbox = np.asarray(inputs["pair_bbox"][sl], dtype=np.float32)
    uf = np.asarray(inputs["union_features"][sl], dtype=np.float32)
    m = {
        "hidx": _wrap_idx(pair_idx[:, 0]),
        "tidx": _wrap_idx(pair_idx[:, 1]),
        "p0w": _wrap_idx(pair_pred[:, 0]),
        "p1w": _wrap_idx(pair_pred[:, 1]),
        "bboxT": np.ascontiguousarray(bbox.T).astype(NPBF16),
        "uT": np.ascontiguousarray(uf.T).astype(NPBF16),
    }
    m.update(common)
    return m


def _prep_common(inputs):
    f32 = lambda k: np.asarray(inputs[k], dtype=np.float32)
    ectx = f32("edge_ctx").astype(NPBF16)

    freqp = np.zeros((NOC * NOC, 128), dtype=np.float32)
    freqp[:, GOFF : GOFF + NRC] = f32("freq_table")
    freqp = freqp.astype(NPBF16)

    wemb = f32("W_post_emb")  # [512, 1024]
    wcat0 = f32("W_post_cat")  # [1024, 4096]
    # fold: ctx_rep @ W_post_cat == [Eh|Et] @ [[Wh@Wcat_top];[Wt@Wcat_bot]]
    wcat = np.concatenate(
        [wemb[:, :H] @ wcat0[:H], wemb[:, H:] @ wcat0[H:]], axis=0
    )  # [1024, 4096]
    wcat_l = np.ascontiguousarray(
        wcat.reshape(KCAT, 128, MO, 128).transpose(2, 1, 0, 3).reshape(MO, 128, KCAT * 128)
    ).astype(NPBF16)

    wspt1_l = f32("W_spt1").astype(NPBF16)  # [32, 512]

    wspt2 = f32("W_spt2")  # [512, 4096]
    wspt2_l = np.ascontiguousarray(
        wspt2.reshape(KC, 128, MO, 128).transpose(2, 1, 0, 3).reshape(MO, 128, KC * 128)
    ).astype(NPBF16)

    wcg = np.zeros((P, 128), dtype=np.float32)
    wcg[:, :NRC] = f32("W_ctx")
    wcg[:, GOFF : GOFF + NRC] = f32("W_gate")
    wcg_l = np.ascontiguousarray(
        wcg.reshape(MO, 128, 128).transpose(1, 0, 2).reshape(128, MO * 128)
    ).astype(NPBF16)

    wvis = np.zeros((P, 128), dtype=np.float32)
    wvis[:, GOFF : GOFF + NRC] = f32("W_vis")
    wvis_l = np.ascontiguousarray(
        wvis.reshape(MO, 128, 128).transpose(1, 0, 2).reshape(128, MO * 128)
    ).astype(NPBF16)

    col = lambda b, n: np.ascontiguousarray(
        np.asarray(b, dtype=np.float32).reshape(n, 128).T
    )
    bctx_l = np.zeros((128, 1), dtype=np.float32)
    bctx_l[:NRC, 0] = f32("b_ctx")
    bvg_l = np.zeros((128, 1), dtype=np.float32)
    bvg_l[GOFF : GOFF + NRC, 0] = f32("b_vis") + f32("b_gate")

    return {
        "ectx": ectx,
        "freqp": freqp,
        "wcat": wcat_l,
        "wspt1": wspt1_l,
        "wspt2": wspt2_l,
        "wcg": wcg_l,
        "wvisp": wvis_l,
        "bcat": col(f32("b_post_emb")[:H] @ wcat0[:H] + f32("b_post_emb")[H:] @ wcat0[H:] + f32("b_post_cat"), MO),
        "bs1": col(inputs["b_spt1"], KC),
        "bs2": col(inputs["b_spt2"], MO),
        "bctx": bctx_l,
        "bvg": bvg_l,
    }


def kernel(**inputs) -> np.ndarray:
    global last_exec_time_ns
    trace = bool(os.environ.get("BASS_KERNEL_TRACE"))
    if trace:
        _register_ntff_hook()
    nc = _build()
    common = _prep_common(inputs)
    in_maps = [_prep_core(inputs, c, common) for c in range(NCORES)]
    res = run_bass_kernel_spmd(nc, in_maps, list(range(NCORES)), trace=trace)
    if trace:
        last_exec_time_ns = res.exec_time_ns
    out = np.concatenate(
        [np.asarray(res.results[c]["out_t"]).T for c in range(NCORES)], axis=0
    )
    return np.ascontiguousarray(out.astype(np.float32))
